# revision 32
# baseline (speedup 1.0000x reference)
"""Trainium2 Bass kernel for DPAttention (attention block + residual + LayerNorm).

Sharding: 8 cores = DP2 (batch) x TP4 (head groups of 3 heads).
Core c: b = c//4, g = c%4 -> heads [3g, 3g+3), output rows [512g, 512g+512) of batch b.

Mask-compaction: the attention mask is a kernel input; invalid keys (mask<0)
contribute exp(-1e9)=0 to softmax, and invalid queries all produce the same
uniform-attention value u = mean_k V (over ALL keys). Host compacts:
  - keys: valid positions only, padded to NKP (mult of 128); pad rows get
    bias -1e9 so e=0.
  - queries: valid positions per 512-row output block, each block padded to
    NQB; 4 blocks -> compact query axis of 4*NQB per batch.
Per-core work drops ~4x on exp/scores/ctx. Invalid output rows use
u = xbar@Wv + bv (xbar = host-computed column mean of X), and the final
out-dense/LN runs on [valid-block | invalid-block] compact rows; the host
scatters rows back.

Per-core dataflow:
  K^T/Q^T [192, *] via matmul(lhsT=W_slice, rhs=X^T-compact), ACT adds bias
  V [NKP, 192] + ones column (softmax denominator for free)
  scores^T [k,q] per (head, block-pair): 64-contraction matmuls, no padding
  e = exp(scores/8 + bias_k) on ScalarE -> bf16
  ctx^T [65, q] accumulated over k-blocks; den row = ctx row 64; normalize
  2 AllGathers (one per block pair) of [192, 2*NQB+1] (last col = u)
  dynamic-offset gather of this core's NQB columns + u -> out dense (valid
  rows) + uo broadcast (invalid rows) + residual + LayerNorm -> [NROW, 768].
"""
import numpy as np
import ml_dtypes

import concourse.bass as bass
import concourse.mybir as mybir
import concourse.tile as tile
from concourse import bacc
from concourse.bass_utils import run_bass_kernel_spmd

F32 = mybir.dt.float32
BF16 = mybir.dt.bfloat16
U32 = mybir.dt.uint32
AF = mybir.ActivationFunctionType
ALU = mybir.AluOpType

B, S, H, NH, HD = 2, 2048, 768, 12, 64
P = 128
KT = H // P            # 6 contraction tiles over hidden
TP = 4                 # head groups (tensor-parallel within a batch)
HG = NH // TP          # 3 heads per core
HGD = HG * HD          # 192
SQ = S // TP           # 512 output rows per core
EPS = 1e-5
SCALE = 1.0 / np.sqrt(HD)
NCORES = 8
GROUPS = [[0, 1, 2, 3], [4, 5, 6, 7]]
BIGNEG = -1.0e9

_cache = {}


def build(NKP, NQB, NROW):
    KB = NKP // P          # key blocks
    NQW = 2 * NQB          # unit width (block pair)
    NQA = 4 * NQB          # full compact query axis
    AGW = NQW              # AllGather payload width
    NST = NROW // P        # LN row tiles
    # out-dense m-blocks covering rows [0, NQB): sizes
    vmbs = []
    r = 0
    while r < NQB:
        vmbs.append((r, min(P, NQB - r)))
        r += P

    nc = bacc.Bacc(num_devices=NCORES)

    xkt_d = nc.dram_tensor("xkt", [H, NKP], BF16, kind="ExternalInput")
    xqt_d = nc.dram_tensor("xqt", [H, NQA], BF16, kind="ExternalInput")
    xbart_d = nc.dram_tensor("xbart", [P, KT], BF16, kind="ExternalInput")
    xres_d = nc.dram_tensor("xres", [NROW, H], F32, kind="ExternalInput")
    wq_d = nc.dram_tensor("wq", [H, HGD], BF16, kind="ExternalInput")
    wk_d = nc.dram_tensor("wk", [H, HGD], BF16, kind="ExternalInput")
    wv_d = nc.dram_tensor("wv", [H, HGD], BF16, kind="ExternalInput")
    wvf_d = nc.dram_tensor("wvf", [H, H], BF16, kind="ExternalInput")
    bq_d = nc.dram_tensor("bq", [HGD], F32, kind="ExternalInput")
    bk_d = nc.dram_tensor("bk", [HGD], F32, kind="ExternalInput")
    bvr_d = nc.dram_tensor("bvr", [P, HGD], F32, kind="ExternalInput")
    bvf_d = nc.dram_tensor("bvf", [P, KT], F32, kind="ExternalInput")
    wo_d = nc.dram_tensor("wo", [H, H], BF16, kind="ExternalInput")
    mkb_d = nc.dram_tensor("mkb", [NKP], F32, kind="ExternalInput")
    lng_d = nc.dram_tensor("lng", [P, H], F32, kind="ExternalInput")
    lnb_d = nc.dram_tensor("lnb", [P, H], F32, kind="ExternalInput")
    xsum_d = nc.dram_tensor("xsum", [NROW], F32, kind="ExternalInput")
    qoff_d = nc.dram_tensor("qoff", [1, 2], U32, kind="ExternalInput")
    out_d = nc.dram_tensor("out", [NROW, H], F32, kind="ExternalOutput")

    with tile.TileContext(nc) as tc:
        with (
            tc.tile_pool(name="wts", bufs=1) as wts,
            tc.tile_pool(name="qkv", bufs=1) as qkv,
            tc.tile_pool(name="dram", bufs=1, space="DRAM") as dram,
        ):
            # ---- load weights / small tensors (attention-critical first) ----
            wk_sb = wts.tile([P, KT, HGD], BF16)
            wq_sb = wts.tile([P, KT, HGD], BF16)
            wv_sb = wts.tile([P, KT, HGD], BF16)
            nc.sync.dma_start(wk_sb[:], wk_d.rearrange("(kt p) d -> p kt d", p=P))
            nc.sync.dma_start(wq_sb[:], wq_d.rearrange("(kt p) d -> p kt d", p=P))
            nc.sync.dma_start(wv_sb[:], wv_d.rearrange("(kt p) d -> p kt d", p=P))

            bq_sb = wts.tile([P, 2], F32)
            bk_sb = wts.tile([P, 2], F32)
            for b_sb, b_d in ((bq_sb, bq_d), (bk_sb, bk_d)):
                nc.gpsimd.dma_start(b_sb[:, 0:1], b_d[0:P].rearrange("(p o) -> p o", o=1))
                nc.gpsimd.dma_start(b_sb[0:HGD - P, 1:2],
                                    b_d[P:HGD].rearrange("(p o) -> p o", o=1))
            mkb_sb = wts.tile([P, KB], F32)
            nc.gpsimd.dma_start(mkb_sb[:], mkb_d.rearrange("(kt p) -> p kt", p=P))
            xbart_sb = wts.tile([P, KT], BF16)
            nc.gpsimd.dma_start(xbart_sb[:], xbart_d[:])
            bvf_sb = wts.tile([P, KT], F32)
            nc.gpsimd.dma_start(bvf_sb[:], bvf_d[:])
            xsum_sb = wts.tile([P, NST], F32)
            nc.gpsimd.dma_start(xsum_sb[:], xsum_d.rearrange("(t p) -> p t", p=P))
            qoff_sb = wts.tile([1, 2], U32)
            nc.gpsimd.dma_start(qoff_sb[:], qoff_d[:])
            bvr_sb = wts.tile([P, HG, HD], F32)
            nc.gpsimd.dma_start(bvr_sb[:], bvr_d.rearrange("p (h d) -> p h d", d=HD))

            # late-needed big tensors on the scalar DMA queue
            wo_sb = wts.tile([P, KT, H], BF16)
            wvf_sb = wts.tile([P, KT, H], BF16)
            xres_sb = wts.tile([P, NST, H], F32)
            lng_sb = wts.tile([P, H], F32)
            lnb_sb = wts.tile([P, H], F32)
            nc.scalar.dma_start(wvf_sb[:], wvf_d.rearrange("(kt p) n -> p kt n", p=P))
            nc.scalar.dma_start(wo_sb[:], wo_d.rearrange("(kt p) n -> p kt n", p=P))
            nc.scalar.dma_start(xres_sb[:], xres_d.rearrange("(t p) n -> p t n", p=P))
            nc.scalar.dma_start(lng_sb[:], lng_d[:])
            nc.scalar.dma_start(lnb_sb[:], lnb_d[:])

            # ---- persistent intermediate tiles ----
            kta_sb = qkv.tile([P, NKP], BF16)    # K^T heads 0,1
            ktb_sb = qkv.tile([HD, NKP], BF16)   # K^T head 2
            qta_sb = qkv.tile([P, NQA], BF16)    # Q^T heads 0,1
            qtb_sb = qkv.tile([HD, NQA], BF16)   # Q^T head 2
            v_sb = qkv.tile([P, KB, HG, HD + 1], BF16)  # V + ones col per head
            ctxa_sb = qkv.tile([P, 2, AGW], BF16)   # ctx^T heads 0,1 per block pair
            ctxb_sb = qkv.tile([HD, 2, AGW], BF16)  # ctx^T head 2
            uoB_sb = qkv.tile([P, H], F32)       # u@Wo broadcast over partitions
            ones_sb = qkv.tile([1, HD], BF16)

            nc.gpsimd.memset(v_sb[:, :, :, HD:HD + 1], 1.0)
            nc.gpsimd.memset(ones_sb[:], 1.0)

            # ================= K/Q projections =====================
            with tc.tile_pool(name="xt", bufs=1) as xtp:
                xkt_sb = xtp.tile([P, KT, NKP], BF16)
                xqt_sb = xtp.tile([P, KT, NQA], BF16)
                xk_r = xkt_d.rearrange("(kt p) s -> p kt s", p=P)
                xq_r = xqt_d.rearrange("(kt p) s -> p kt s", p=P)
                for kt in range(KT):
                    nc.sync.dma_start(xkt_sb[:, kt, :], xk_r[:, kt, :])
                for kt in range(KT):
                    nc.sync.dma_start(xqt_sb[:, kt, :], xq_r[:, kt, :])

                # ============= attention (software-pipelined units) =============
                units = [(h, bp) for bp in range(2) for h in range(HG)]

                from contextlib import ExitStack
                attn_ctx = ExitStack()
                epool = attn_ctx.enter_context(tc.tile_pool(name="epool", bufs=2))
                sps = attn_ctx.enter_context(tc.tile_pool(name="sps", bufs=2, space="PSUM"))
                cps = attn_ctx.enter_context(tc.tile_pool(name="cps", bufs=2, space="PSUM"))
                npool = attn_ctx.enter_context(tc.tile_pool(name="npool", bufs=2))

                def emit_proj_piece(x_sb, w_sb, b_sb, da, db, width, mp, qc):
                    m0, msz = (0, P) if mp == 0 else (P, HGD - P)
                    q0 = qc * 512
                    qsz = min(512, width - q0)
                    ps = sps.tile([P, 2, 512], F32, tag="sc",
                                  name=f"pj{mp}_{qc}_{id(w_sb)%97}")
                    for kt in range(KT):
                        nc.tensor.matmul(
                            ps[:msz, 0, 0:qsz],
                            w_sb[:, kt, m0:m0 + msz],
                            x_sb[:, kt, q0:q0 + qsz],
                            start=(kt == 0), stop=(kt == KT - 1),
                        )
                    dst = (da[:, q0:q0 + qsz] if mp == 0
                           else db[:, q0:q0 + qsz])
                    nc.scalar.activation(dst, ps[:msz, 0, 0:qsz], AF.Identity,
                                         bias=b_sb[:msz, mp:mp + 1])

                NCK = (NKP + 511) // 512
                NCQ = (NQA + 511) // 512
                # pre-attention: only what unit (0,0)/(1,0) need (mp0 rows)
                for qc in range(NCK):
                    emit_proj_piece(xkt_sb, wk_sb, bk_sb, kta_sb, ktb_sb, NKP, 0, qc)
                for qc in range(min(2, NCQ)):
                    emit_proj_piece(xqt_sb, wq_sb, bq_sb, qta_sb, qtb_sb, NQA, 0, qc)
                # remaining pieces, interleaved into unit 0's kb loop
                proj_rest = (
                    [(xkt_sb, wk_sb, bk_sb, kta_sb, ktb_sb, NKP, 1, qc)
                     for qc in range(NCK)]
                    + [(xqt_sb, wq_sb, bq_sb, qta_sb, qtb_sb, NQA, 0, qc)
                       for qc in range(2, NCQ)]
                    + [(xqt_sb, wq_sb, bq_sb, qta_sb, qtb_sb, NQA, 1, qc)
                       for qc in range(NCQ)]
                )

                e_tiles = {}
                c_tiles = {}

                def kt_of(h):
                    return (kta_sb[HD * h:HD * (h + 1), :] if h < 2
                            else ktb_sb[:, :])

                def qt_of(h):
                    return (qta_sb[HD * h:HD * (h + 1), :] if h < 2
                            else qtb_sb[:, :])

                def emit_scores_kb(i, kb):
                    h, bp = units[i]
                    e_t = e_tiles[i]
                    ps = sps.tile([P, 2, 512], F32, tag="sc", name=f"sc{i}_{kb}")
                    lhsT = kt_of(h)[:, kb * P:(kb + 1) * P]
                    for sub in range(2):
                        q0 = bp * NQW + sub * NQB
                        nc.tensor.matmul(ps[:, sub, 0:NQB], lhsT,
                                         qt_of(h)[:, q0:q0 + NQB],
                                         start=True, stop=True)
                    nc.scalar.activation(
                        e_t[:, kb, :].rearrange("p (s q) -> p s q", q=NQB),
                        ps[:, :, 0:NQB], AF.Exp,
                        bias=mkb_sb[:, kb:kb + 1], scale=float(SCALE))

                def emit_vproj(kb):
                    ps = cps.tile([P, HGD], F32, tag="c", name=f"vp{kb}")
                    for kt in range(KT):
                        nc.tensor.matmul(
                            ps[:], xkt_sb[:, kt, kb * P:(kb + 1) * P], wv_sb[:, kt, :],
                            start=(kt == 0), stop=(kt == KT - 1),
                        )
                    nc.vector.tensor_tensor(
                        v_sb[:, kb, :, 0:HD], ps[:].rearrange("p (h d) -> p h d", d=HD),
                        bvr_sb[:], op=ALU.add,
                    )

                def emit_uo():
                    # u^T [768,1] = Wv^T @ xbar^T + bv  (full hidden, local)
                    ups = cps.tile([P, KT], F32, tag="c", name="ups")
                    for mb in range(KT):
                        for kt in range(KT):
                            nc.tensor.matmul(
                                ups[:, mb:mb + 1],
                                wvf_sb[:, kt, mb * P:(mb + 1) * P],
                                xbart_sb[:, kt:kt + 1],
                                start=(kt == 0), stop=(kt == KT - 1),
                            )
                    ut_sb = npool.tile([P, KT], BF16, tag="ut", bufs=1)
                    utf = npool.tile([P, KT], F32, tag="utf", bufs=1)
                    nc.vector.tensor_tensor(utf[:], ups[:], bvf_sb[:], op=ALU.add)
                    nc.vector.tensor_copy(ut_sb[:], utf[:])
                    # uo^T [768,1] = Wo^T @ u^T
                    uops = cps.tile([P, KT], F32, tag="c", name="uops")
                    for mb in range(KT):
                        for kt in range(KT):
                            nc.tensor.matmul(
                                uops[:, mb:mb + 1],
                                wo_sb[:, kt, mb * P:(mb + 1) * P],
                                ut_sb[:, kt:kt + 1],
                                start=(kt == 0), stop=(kt == KT - 1),
                            )
                    uot = npool.tile([P, KT], F32, tag="uot", bufs=1)
                    nc.vector.tensor_copy(uot[:], uops[:])
                    uod = dram.tile([1, H], F32, name="uod")
                    nc.sync.dma_start(
                        uod.rearrange("o (kt p) -> p kt o", p=P), uot[:])
                    nc.sync.dma_start(uoB_sb[:], uod[0:1, :].to_broadcast((P, H)))

                def emit_ctx_kb(i, kb):
                    h, bp = units[i]
                    e_t = e_tiles[i]
                    pc = c_tiles[i]
                    for sub in range(2):
                        nc.tensor.matmul(
                            pc[0:HD + 1, sub, 0:NQB], v_sb[:, kb, h, :],
                            e_t[:, kb, sub * NQB:(sub + 1) * NQB],
                            start=(kb == 0), stop=(kb == KB - 1),
                        )

                def emit_ctx_tail(i):
                    h, bp = units[i]
                    pc = c_tiles[i]
                    # den row -> bf16 SBUF (ACT), broadcast to 64 partitions on
                    # the PE (ones ⊗ den), reciprocal + multiply on 64-lane DVE.
                    denb = npool.tile([1, NQW], BF16, tag="den")
                    nc.scalar.activation(
                        denb[:].rearrange("p (s q) -> p s q", q=NQB),
                        pc[HD:HD + 1, :, 0:NQB], AF.Copy)
                    rbps = sps.tile([P, 2, 512], F32, tag="sc", name=f"rb{i}")
                    for sub in range(2):
                        nc.tensor.matmul(rbps[0:HD, sub, 0:NQB], ones_sb[:],
                                         denb[:, sub * NQB:(sub + 1) * NQB],
                                         start=True, stop=True)
                    rb = npool.tile([HD, NQW], F32, tag="rb")
                    rb3 = rb[:].rearrange("p (s q) -> p s q", q=NQB)
                    nc.vector.reciprocal_approx_fast(rb3, rbps[0:HD, :, 0:NQB])
                    dst = (ctxa_sb[HD * h:HD * (h + 1), bp, 0:NQW] if h < 2
                           else ctxb_sb[:, bp, 0:NQW])
                    nc.vector.tensor_tensor(
                        dst.rearrange("p (s q) -> p s q", q=NQB),
                        pc[0:HD, :, 0:NQB], rb3, op=ALU.mult)

                ag_in = [dram.tile([HGD, NQW], BF16, name="agi0"),
                         dram.tile([HGD, NQW], BF16, name="agi1")]
                ag_out = dram.tile([2, TP, HGD, NQW], BF16)

                def emit_ag(bp):
                    # distinct DMA queues per AG -> independent completion
                    # semaphores, so AG(0) can start mid-attention instead of
                    # serializing both AGs at attention end.
                    dq = nc.scalar if bp == 0 else nc.sync
                    dq.dma_start(ag_in[bp][0:P, :], ctxa_sb[:, bp, :])
                    dq.dma_start(ag_in[bp][P:HGD, :], ctxb_sb[:, bp, :])
                    with nc.named_scope(f"ag{bp}"):
                        nc.gpsimd.collective_compute(
                            "AllGather", ALU.bypass, replica_groups=GROUPS,
                            ins=[ag_in[bp].opt()], outs=[ag_out[bp].opt()],
                        )

                for i in range(len(units) + 1):
                    if i < len(units):
                        e_tiles[i] = epool.tile([P, KB, NQW], BF16, tag="e",
                                                name=f"e{i}")
                    if i > 0:
                        c_tiles[i - 1] = cps.tile([P, 2, 512], F32, tag="c",
                                                  name=f"c{i-1}")
                    with nc.named_scope(f"unit{i}"):
                        for kb in range(KB):
                            if i < len(units):
                                emit_scores_kb(i, kb)
                            if i == 0:
                                emit_vproj(kb)
                                if kb < len(proj_rest):
                                    emit_proj_piece(*proj_rest[kb])
                            if i > 0:
                                emit_ctx_kb(i - 1, kb)
                    if i == 0:
                        emit_uo()
                    if i > 0:
                        emit_ctx_tail(i - 1)
                        del e_tiles[i - 1]
                        if (i - 1) % HG == HG - 1:
                            emit_ag((i - 1) // HG)
                attn_ctx.close()

            # ================= gather ctx for this core's block =================
            with tc.tile_critical():
                with nc.gpsimd.register("qx") as qx_reg:
                    nc.gpsimd.reg_load(qx_reg, qoff_sb[0:1, 0:1])
                    qx_v = nc.gpsimd.snap(qx_reg)
                with nc.gpsimd.register("qi") as qi_reg:
                    nc.gpsimd.reg_load(qi_reg, qoff_sb[0:1, 1:2])
                    qi_v = nc.gpsimd.snap(qi_reg)

            ag_r = (ag_out.rearrange("x g d q -> x (g d) q")
                    .rearrange("x (kt p) q -> p kt x q", p=P))
            ctxg_sb = qkv.tile([P, KT, NQB], BF16)
            with nc.named_scope("gather"):
                nc.gpsimd.dma_start(
                    ctxg_sb[:], ag_r[:, :, bass.ds(qx_v, 1), bass.ds(qi_v, NQB)])

            # ================= out dense + residual + LayerNorm =================
            with tc.tile_pool(name="ops", bufs=2, space="PSUM") as ops, \
                 tc.tile_pool(name="lnp", bufs=1) as lnp:
                uoB = uoB_sb
                h_all = lnp.tile([P, NST, H], F32)
                mu_all = lnp.tile([P, NST], F32)
                var_all = lnp.tile([P, NST], F32)
                sq_tmp = lnp.tile([P, H], F32, bufs=2)
                for st in range(NST):
                    r0 = st * P
                    vsz = max(0, min(P, NQB - r0))   # valid-dense rows in this tile
                    if vsz > 0:
                        ps = ops.tile([P, H], F32, tag="od", name=f"od{st}")
                        for kt in range(KT):
                            lhsT = ctxg_sb[:, kt, r0:r0 + vsz]
                            nc.tensor.matmul(ps[0:vsz, 0:512], lhsT,
                                             wo_sb[:, kt, 0:512],
                                             start=(kt == 0), stop=(kt == KT - 1))
                            nc.tensor.matmul(ps[0:vsz, 512:H], lhsT,
                                             wo_sb[:, kt, 512:H],
                                             start=(kt == 0), stop=(kt == KT - 1))
                        psc = lnp.tile([P, H], F32, tag="psc", bufs=2,
                                       name=f"psc{st}")
                        nc.scalar.activation(psc[0:vsz], ps[0:vsz], AF.Identity,
                                             accum_out=mu_all[0:vsz, st:st + 1])
                        nc.vector.tensor_tensor(h_all[0:vsz, st, :], psc[0:vsz],
                                                xres_sb[0:vsz, st, :], op=ALU.add)
                    # invalid-query rows: h = xres + uo (partition-aligned chunks)
                    s = vsz
                    while s < P:
                        m = 128 if s == 0 else (64 if s % 64 == 0 else 32)
                        e = min(s + m, P)
                        nc.vector.scalar_tensor_tensor(
                            out=h_all[s:e, st, :], in0=xres_sb[s:e, st, :],
                            scalar=1.0, in1=uoB[s:e, :],
                            op0=ALU.mult, op1=ALU.add,
                            accum_out=mu_all[s:e, st:st + 1])
                        s = e

                # mu = (accum + xsum)/H ; var via ACT Square-accum
                nc.vector.tensor_tensor(mu_all[:], mu_all[:], xsum_sb[:], op=ALU.add)
                nc.vector.tensor_scalar_mul(mu_all[:], mu_all[:], 1.0 / H)
                negmu = lnp.tile([P, NST], F32)
                nc.vector.tensor_scalar_mul(negmu[:], mu_all[:], -1.0)
                for st in range(NST):
                    nc.scalar.activation(sq_tmp[:], h_all[:, st, :], AF.Square,
                                         bias=negmu[:, st:st + 1],
                                         accum_out=var_all[:, st:st + 1])
                nc.vector.tensor_scalar_mul(var_all[:], var_all[:], 1.0 / H)
                nc.vector.tensor_scalar_add(var_all[:], var_all[:], EPS)
                # rstd = 1/sqrt(var) with one Newton step
                std0 = lnp.tile([P, NST], F32)
                nc.scalar.activation(std0[:], var_all[:], AF.Sqrt)
                y0 = lnp.tile([P, NST], F32)
                nc.vector.reciprocal(y0[:], std0[:])
                t0 = lnp.tile([P, NST], F32)
                nc.vector.tensor_tensor(t0[:], y0[:], y0[:], op=ALU.mult)
                nc.vector.tensor_tensor(t0[:], t0[:], var_all[:], op=ALU.mult)
                nc.vector.tensor_scalar_mul(t0[:], t0[:], -0.5)
                nc.vector.tensor_scalar_add(t0[:], t0[:], 1.5)
                rstd = lnp.tile([P, NST], F32)
                nc.vector.tensor_tensor(rstd[:], y0[:], t0[:], op=ALU.mult)

                for st in range(NST):
                    hc = lnp.tile([P, H], F32, tag="hc", bufs=2, name=f"hc{st}")
                    nc.scalar.activation(hc[:], h_all[:, st, :], AF.Identity,
                                         bias=negmu[:, st:st + 1])
                    o_sb = lnp.tile([P, H], F32, tag="o", bufs=2, name=f"o{st}")
                    nc.vector.scalar_tensor_tensor(
                        out=o_sb[:], in0=hc[:], scalar=rstd[:, st:st + 1],
                        in1=lng_sb[:], op0=ALU.mult, op1=ALU.mult)
                    nc.vector.tensor_tensor(o_sb[:], o_sb[:], lnb_sb[:], op=ALU.add)
                    nc.sync.dma_start(out_d[st * P:(st + 1) * P, :], o_sb[:])

    nc.compile()
    return nc


def _geometry(am):
    valid = am >= 0
    vidx = [np.where(valid[b])[0] for b in range(B)]
    NKP = int(-(-max(len(v) for v in vidx) // P) * P)
    bidx = {}
    iidx = {}
    for b in range(B):
        for g in range(TP):
            lo, hi = SQ * g, SQ * (g + 1)
            m = (vidx[b] >= lo) & (vidx[b] < hi)
            bidx[(b, g)] = vidx[b][m]
            inv = np.where(~valid[b, lo:hi])[0] + lo
            iidx[(b, g)] = inv
    maxv = max(len(v) for v in bidx.values())
    maxi = max(len(v) for v in iidx.values())
    NQB = int(-(-max(maxv, 1) // 32) * 32)
    need = NQB + int(-(-max(maxi, 1) // 32) * 32)
    NROW = int(-(-need // P) * P)
    return vidx, bidx, iidx, NKP, NQB, NROW


def _prep_inputs(inputs, geom):
    vidx, bidx, iidx, NKP, NQB, NROW = geom
    NQA = 4 * NQB
    hs = np.asarray(inputs["hidden_states"], dtype=np.float32)
    Wq = np.asarray(inputs["Wq"], dtype=np.float32)
    Wk = np.asarray(inputs["Wk"], dtype=np.float32)
    Wv = np.asarray(inputs["Wv"], dtype=np.float32)
    Wo = np.asarray(inputs["Wo"], dtype=np.float32)
    bq = np.asarray(inputs["bq"], dtype=np.float32)
    bk = np.asarray(inputs["bk"], dtype=np.float32)
    bv = np.asarray(inputs["bv"], dtype=np.float32)
    bo = np.asarray(inputs["bo"], dtype=np.float32)
    lng = np.asarray(inputs["ln_gamma"], dtype=np.float32)
    lnb = np.asarray(inputs["ln_beta"], dtype=np.float32)

    wo_bf = Wo.astype(ml_dtypes.bfloat16)
    wv_bf = Wv.astype(ml_dtypes.bfloat16)
    bvf_rep = np.ascontiguousarray(bv.reshape(KT, P).T.astype(np.float32))
    lng_rep = np.ascontiguousarray(np.broadcast_to(lng, (P, H)))
    lnb_rep = np.ascontiguousarray(np.broadcast_to(lnb, (P, H)))

    # per-batch compacted tensors
    xkt = []
    xqt = []
    xbart = []
    mkb = []
    for b in range(B):
        xk = np.zeros((H, NKP), dtype=ml_dtypes.bfloat16)
        xk[:, :len(vidx[b])] = hs[b].T[:, vidx[b]].astype(ml_dtypes.bfloat16)
        xkt.append(xk)
        xq = np.zeros((H, NQA), dtype=ml_dtypes.bfloat16)
        for g in range(TP):
            bi = bidx[(b, g)]
            xq[:, NQB * g:NQB * g + len(bi)] = \
                hs[b].T[:, bi].astype(ml_dtypes.bfloat16)
        xqt.append(xq)
        xbart.append(np.ascontiguousarray(
            hs[b].mean(axis=0).reshape(KT, P).T.astype(ml_dtypes.bfloat16)))
        mk = np.zeros(NKP, dtype=np.float32)
        mk[len(vidx[b]):] = BIGNEG
        mkb.append(mk)

    in_maps = []
    for c in range(NCORES):
        b, g = c // TP, c % TP
        cs = slice(HGD * g, HGD * (g + 1))
        bi = bidx[(b, g)]
        ii = iidx[(b, g)]
        xres = np.zeros((NROW, H), dtype=np.float32)
        xres[0:len(bi)] = hs[b, bi] + bo
        xres[NQB:NQB + len(ii)] = hs[b, ii] + bo
        xsum = np.zeros(NROW, dtype=np.float32)
        xsum[0:len(bi)] = xres[0:len(bi)].sum(axis=1)
        in_maps.append({
            "xkt": xkt[b],
            "xqt": xqt[b],
            "xbart": xbart[b],
            "xres": xres,
            "wq": np.ascontiguousarray(Wq[:, cs]).astype(ml_dtypes.bfloat16),
            "wk": np.ascontiguousarray(Wk[:, cs]).astype(ml_dtypes.bfloat16),
            "wv": np.ascontiguousarray(Wv[:, cs]).astype(ml_dtypes.bfloat16),
            "bq": np.ascontiguousarray(bq[cs]),
            "bk": np.ascontiguousarray(bk[cs]),
            "bvr": np.ascontiguousarray(np.broadcast_to(bv[cs], (P, HGD))),
            "wvf": wv_bf,
            "bvf": bvf_rep,
            "wo": np.ascontiguousarray(wo_bf),
            "mkb": mkb[b],
            "lng": lng_rep,
            "lnb": lnb_rep,
            "xsum": xsum,
            "qoff": np.array([[g // 2, (g % 2) * NQB]], dtype=np.uint32),
        })
    return in_maps


def _run(inputs, trace=False, trace_cores=None):
    am = np.asarray(inputs["attention_mask"], dtype=np.float32)
    geom = _geometry(am)
    _, bidx, iidx, NKP, NQB, NROW = geom
    key = (NKP, NQB, NROW)
    if key not in _cache:
        _cache[key] = build(*key)
    nc = _cache[key]
    in_maps = _prep_inputs(inputs, geom)
    res = run_bass_kernel_spmd(
        nc, in_maps, list(range(NCORES)), trace=trace,
        trace_cores=trace_cores,
    )
    out = np.empty((B, S, H), dtype=np.float32)
    for c in range(NCORES):
        b, g = c // TP, c % TP
        r = res.results[c]["out"]
        bi = bidx[(b, g)]
        ii = iidx[(b, g)]
        out[b, bi] = r[0:len(bi)]
        out[b, ii] = r[NQB:NQB + len(ii)]
    return out, res


def kernel(**inputs) -> np.ndarray:
    out, _ = _run(inputs)
    return out


# revision 43
# speedup vs baseline: 1.2661x; 1.2661x over previous
"""Trainium2 Bass kernel for DPAttention (attention block + residual + LayerNorm).

Sharding: 8 cores = DP2 (batch) x TP4 (head groups of 3 heads).
Core c: b = c//4, g = c%4 -> heads [3g, 3g+3), output rows [512g, 512g+512) of batch b.

Mask-compaction: the attention mask is a kernel input; invalid keys (mask<0)
contribute exp(-1e9)=0 to softmax, and invalid queries all produce the same
uniform-attention value u = mean_k V (over ALL keys). Host compacts:
  - keys: valid positions only, padded to NKP (mult of 128); pad rows get
    bias -1e9 so e=0.
  - queries: valid positions per 512-row output block, each block padded to
    NQB; 4 blocks -> compact query axis of 4*NQB per batch.
Per-core work drops ~4x on exp/scores/ctx. Invalid output rows use
u = xbar@Wv + bv (xbar = host-computed column mean of X), and the final
out-dense/LN runs on [valid-block | invalid-block] compact rows; the host
scatters rows back.

Per-core dataflow:
  K^T/Q^T [192, *] via matmul(lhsT=W_slice, rhs=X^T-compact), ACT adds bias
  V [NKP, 192] + ones column (softmax denominator for free)
  scores^T [k,q] per (head, block-pair): 64-contraction matmuls, no padding
  e = exp(scores/8 + bias_k) on ScalarE -> bf16
  ctx^T [65, q] accumulated over k-blocks; den row = ctx row 64; normalize
  2 AllGathers (one per block pair) of [192, 2*NQB+1] (last col = u)
  dynamic-offset gather of this core's NQB columns + u -> out dense (valid
  rows) + uo broadcast (invalid rows) + residual + LayerNorm -> [NROW, 768].
"""
import numpy as np
import ml_dtypes

import concourse.bass as bass
import concourse.mybir as mybir
import concourse.tile as tile
from concourse import bacc
from concourse.bass_utils import run_bass_kernel_spmd

F32 = mybir.dt.float32
BF16 = mybir.dt.bfloat16
U32 = mybir.dt.uint32
AF = mybir.ActivationFunctionType
ALU = mybir.AluOpType

B, S, H, NH, HD = 2, 2048, 768, 12, 64
P = 128
KT = H // P            # 6 contraction tiles over hidden
TP = 4                 # head groups (tensor-parallel within a batch)
HG = NH // TP          # 3 heads per core
HGD = HG * HD          # 192
SQ = S // TP           # 512 output rows per core
EPS = 1e-5
SCALE = 1.0 / np.sqrt(HD)
NCORES = 8
GROUPS = [[0, 1, 2, 3], [4, 5, 6, 7]]
BIGNEG = -1.0e9

_cache = {}


def build(NQB, NROW):
    NQA = 4 * NQB          # full compact axis (block-padded valid positions)
    NKP = NQA              # keys = same block-padded compact axis
    KB = NKP // P          # key blocks
    NQW = 2 * NQB          # unit width (block pair)
    AGW = NQW              # AllGather payload width
    NST = NROW // P        # LN row tiles
    # out-dense m-blocks covering rows [0, NQB): sizes
    vmbs = []
    r = 0
    while r < NQB:
        vmbs.append((r, min(P, NQB - r)))
        r += P

    nc = bacc.Bacc(num_devices=NCORES)

    xkt_d = nc.dram_tensor("xkt", [H, NKP], BF16, kind="ExternalInput")
    xbart_d = nc.dram_tensor("xbart", [P, KT], BF16, kind="ExternalInput")
    xres_d = nc.dram_tensor("xres", [NROW, H], F32, kind="ExternalInput")
    wq_d = nc.dram_tensor("wq", [H, HGD], BF16, kind="ExternalInput")
    wk_d = nc.dram_tensor("wk", [H, HGD], BF16, kind="ExternalInput")
    wv_d = nc.dram_tensor("wv", [H, HGD], BF16, kind="ExternalInput")
    wvf_d = nc.dram_tensor("wvf", [H, H], BF16, kind="ExternalInput")
    bq_d = nc.dram_tensor("bq", [HGD], F32, kind="ExternalInput")
    bk_d = nc.dram_tensor("bk", [HGD], F32, kind="ExternalInput")
    bvr_d = nc.dram_tensor("bvr", [P, HGD], F32, kind="ExternalInput")
    bvf_d = nc.dram_tensor("bvf", [P, KT], F32, kind="ExternalInput")
    wo_d = nc.dram_tensor("wo", [H, H], BF16, kind="ExternalInput")
    mkb_d = nc.dram_tensor("mkb", [NKP], F32, kind="ExternalInput")
    lng_d = nc.dram_tensor("lng", [P, H], F32, kind="ExternalInput")
    lnb_d = nc.dram_tensor("lnb", [P, H], F32, kind="ExternalInput")
    xsum_d = nc.dram_tensor("xsum", [NROW], F32, kind="ExternalInput")
    qoff_d = nc.dram_tensor("qoff", [1, 2], U32, kind="ExternalInput")
    out_d = nc.dram_tensor("out", [NROW, H], F32, kind="ExternalOutput")

    with tile.TileContext(nc) as tc:
        with (
            tc.tile_pool(name="wts", bufs=1) as wts,
            tc.tile_pool(name="qkv", bufs=1) as qkv,
            tc.tile_pool(name="dram", bufs=1, space="DRAM") as dram,
        ):
            # ---- load weights / small tensors (attention-critical first) ----
            wk_sb = wts.tile([P, KT, HGD], BF16)
            wq_sb = wts.tile([P, KT, HGD], BF16)
            wv_sb = wts.tile([P, KT, HGD], BF16)
            nc.sync.dma_start(wk_sb[:], wk_d.rearrange("(kt p) d -> p kt d", p=P))
            nc.sync.dma_start(wq_sb[:], wq_d.rearrange("(kt p) d -> p kt d", p=P))
            nc.sync.dma_start(wv_sb[:], wv_d.rearrange("(kt p) d -> p kt d", p=P))

            bq_sb = wts.tile([P, 2], F32)
            bk_sb = wts.tile([P, 2], F32)
            for b_sb, b_d in ((bq_sb, bq_d), (bk_sb, bk_d)):
                nc.gpsimd.dma_start(b_sb[:, 0:1], b_d[0:P].rearrange("(p o) -> p o", o=1))
                nc.gpsimd.dma_start(b_sb[0:HGD - P, 1:2],
                                    b_d[P:HGD].rearrange("(p o) -> p o", o=1))
            mkb_sb = wts.tile([P, KB], F32)
            nc.gpsimd.dma_start(mkb_sb[:], mkb_d.rearrange("(kt p) -> p kt", p=P))
            xbart_sb = wts.tile([P, KT], BF16)
            nc.gpsimd.dma_start(xbart_sb[:], xbart_d[:])
            bvf_sb = wts.tile([P, KT], F32)
            nc.gpsimd.dma_start(bvf_sb[:], bvf_d[:])
            xsum_sb = wts.tile([P, NST], F32)
            nc.gpsimd.dma_start(xsum_sb[:], xsum_d.rearrange("(t p) -> p t", p=P))
            qoff_sb = wts.tile([1, 2], U32)
            nc.gpsimd.dma_start(qoff_sb[:], qoff_d[:])
            bvr_sb = wts.tile([P, HG, HD], F32)
            nc.gpsimd.dma_start(bvr_sb[:], bvr_d.rearrange("p (h d) -> p h d", d=HD))

            # late-needed big tensors on the scalar DMA queue
            wo_sb = wts.tile([P, KT, H], BF16)
            wvf_sb = wts.tile([P, KT, H], BF16)
            xres_sb = wts.tile([P, NST, H], F32)
            lng_sb = wts.tile([P, H], F32)
            lnb_sb = wts.tile([P, H], F32)
            nc.scalar.dma_start(wvf_sb[:], wvf_d.rearrange("(kt p) n -> p kt n", p=P))
            nc.scalar.dma_start(wo_sb[:], wo_d.rearrange("(kt p) n -> p kt n", p=P))
            nc.scalar.dma_start(xres_sb[:], xres_d.rearrange("(t p) n -> p t n", p=P))
            nc.scalar.dma_start(lng_sb[:], lng_d[:])
            nc.scalar.dma_start(lnb_sb[:], lnb_d[:])

            # ---- persistent intermediate tiles ----
            kta_sb = qkv.tile([P, NKP], BF16)    # K^T heads 0,1
            ktb_sb = qkv.tile([HD, NKP], BF16)   # K^T head 2
            qta_sb = qkv.tile([P, NQA], BF16)    # Q^T heads 0,1
            qtb_sb = qkv.tile([HD, NQA], BF16)   # Q^T head 2
            v_sb = qkv.tile([P, KB, HG, HD + 1], BF16)  # V + ones col per head
            ctxa_sb = qkv.tile([P, 2, AGW], BF16)   # ctx^T heads 0,1 per block pair
            ctxb_sb = qkv.tile([HD, 2, AGW], BF16)  # ctx^T head 2
            uoB_sb = qkv.tile([P, H], F32)       # u@Wo broadcast over partitions
            ones_sb = qkv.tile([1, HD], BF16)

            nc.gpsimd.memset(v_sb[:, :, :, HD:HD + 1], 1.0)
            nc.gpsimd.memset(ones_sb[:], 1.0)

            # ================= K/Q projections =====================
            with tc.tile_pool(name="xt", bufs=1) as xtp:
                xkt_sb = xtp.tile([P, KT, NKP], BF16)
                xk_r = xkt_d.rearrange("(kt p) s -> p kt s", p=P)
                for kt in range(KT):
                    nc.sync.dma_start(xkt_sb[:, kt, :], xk_r[:, kt, :])
                xqt_sb = xkt_sb  # queries = same block-padded compact axis

                # ============= attention (software-pipelined units) =============
                units = [(h, bp) for bp in range(2) for h in range(HG)]

                from contextlib import ExitStack
                attn_ctx = ExitStack()
                epool = attn_ctx.enter_context(tc.tile_pool(name="epool", bufs=2))
                sps = attn_ctx.enter_context(tc.tile_pool(name="sps", bufs=2, space="PSUM"))
                cps = attn_ctx.enter_context(tc.tile_pool(name="cps", bufs=2, space="PSUM"))
                npool = attn_ctx.enter_context(tc.tile_pool(name="npool", bufs=2))

                def emit_proj_piece(x_sb, w_sb, b_sb, da, db, width, mp, qc):
                    m0, msz = (0, P) if mp == 0 else (P, HGD - P)
                    q0 = qc * 512
                    qsz = min(512, width - q0)
                    ps = sps.tile([P, 2, 512], F32, tag="sc",
                                  name=f"pj{mp}_{qc}_{id(w_sb)%97}")
                    for kt in range(KT):
                        nc.tensor.matmul(
                            ps[:msz, 0, 0:qsz],
                            w_sb[:, kt, m0:m0 + msz],
                            x_sb[:, kt, q0:q0 + qsz],
                            start=(kt == 0), stop=(kt == KT - 1),
                        )
                    dst = (da[:, q0:q0 + qsz] if mp == 0
                           else db[:, q0:q0 + qsz])
                    nc.scalar.activation(dst, ps[:msz, 0, 0:qsz], AF.Identity,
                                         bias=b_sb[:msz, mp:mp + 1])

                NCK = (NKP + 511) // 512
                NCQ = (NQA + 511) // 512
                # pre-attention: only what unit (0,0)/(1,0) need (mp0 rows)
                for qc in range(NCK):
                    emit_proj_piece(xkt_sb, wk_sb, bk_sb, kta_sb, ktb_sb, NKP, 0, qc)
                for qc in range(min(2, NCQ)):
                    emit_proj_piece(xqt_sb, wq_sb, bq_sb, qta_sb, qtb_sb, NQA, 0, qc)
                # remaining pieces, interleaved into unit 0's kb loop
                proj_rest = (
                    [(xkt_sb, wk_sb, bk_sb, kta_sb, ktb_sb, NKP, 1, qc)
                     for qc in range(NCK)]
                    + [(xqt_sb, wq_sb, bq_sb, qta_sb, qtb_sb, NQA, 0, qc)
                       for qc in range(2, NCQ)]
                    + [(xqt_sb, wq_sb, bq_sb, qta_sb, qtb_sb, NQA, 1, qc)
                       for qc in range(NCQ)]
                )

                e_tiles = {}
                c_tiles = {}

                def kt_of(h):
                    return (kta_sb[HD * h:HD * (h + 1), :] if h < 2
                            else ktb_sb[:, :])

                def qt_of(h):
                    return (qta_sb[HD * h:HD * (h + 1), :] if h < 2
                            else qtb_sb[:, :])

                def emit_scores_kb(i, kb):
                    h, bp = units[i]
                    e_t = e_tiles[i]
                    ps = sps.tile([P, 2, 512], F32, tag="sc", name=f"sc{i}_{kb}")
                    lhsT = kt_of(h)[:, kb * P:(kb + 1) * P]
                    for sub in range(2):
                        q0 = bp * NQW + sub * NQB
                        nc.tensor.matmul(ps[:, sub, 0:NQB], lhsT,
                                         qt_of(h)[:, q0:q0 + NQB],
                                         start=True, stop=True)
                    nc.scalar.activation(
                        e_t[:, kb, :].rearrange("p (s q) -> p s q", q=NQB),
                        ps[:, :, 0:NQB], AF.Exp,
                        bias=mkb_sb[:, kb:kb + 1], scale=float(SCALE))

                def emit_vproj(kb):
                    ps = cps.tile([P, HGD], F32, tag="c", name=f"vp{kb}")
                    for kt in range(KT):
                        nc.tensor.matmul(
                            ps[:], xkt_sb[:, kt, kb * P:(kb + 1) * P], wv_sb[:, kt, :],
                            start=(kt == 0), stop=(kt == KT - 1),
                        )
                    nc.vector.tensor_tensor(
                        v_sb[:, kb, :, 0:HD], ps[:].rearrange("p (h d) -> p h d", d=HD),
                        bvr_sb[:], op=ALU.add,
                    )

                def emit_uo():
                    # u^T [768,1] = Wv^T @ xbar^T + bv  (full hidden, local)
                    ups = cps.tile([P, KT], F32, tag="c", name="ups")
                    for mb in range(KT):
                        for kt in range(KT):
                            nc.tensor.matmul(
                                ups[:, mb:mb + 1],
                                wvf_sb[:, kt, mb * P:(mb + 1) * P],
                                xbart_sb[:, kt:kt + 1],
                                start=(kt == 0), stop=(kt == KT - 1),
                            )
                    ut_sb = npool.tile([P, KT], BF16, tag="ut", bufs=1)
                    utf = npool.tile([P, KT], F32, tag="utf", bufs=1)
                    nc.vector.tensor_tensor(utf[:], ups[:], bvf_sb[:], op=ALU.add)
                    nc.vector.tensor_copy(ut_sb[:], utf[:])
                    # uo^T [768,1] = Wo^T @ u^T
                    uops = cps.tile([P, KT], F32, tag="c", name="uops")
                    for mb in range(KT):
                        for kt in range(KT):
                            nc.tensor.matmul(
                                uops[:, mb:mb + 1],
                                wo_sb[:, kt, mb * P:(mb + 1) * P],
                                ut_sb[:, kt:kt + 1],
                                start=(kt == 0), stop=(kt == KT - 1),
                            )
                    uot = npool.tile([P, KT], F32, tag="uot", bufs=1)
                    nc.vector.tensor_copy(uot[:], uops[:])
                    uod = dram.tile([1, H], F32, name="uod")
                    nc.sync.dma_start(
                        uod.rearrange("o (kt p) -> p kt o", p=P), uot[:])
                    nc.sync.dma_start(uoB_sb[:], uod[0:1, :].to_broadcast((P, H)))

                def emit_ctx_kb(i, kb):
                    h, bp = units[i]
                    e_t = e_tiles[i]
                    pc = c_tiles[i]
                    for sub in range(2):
                        nc.tensor.matmul(
                            pc[0:HD + 1, sub, 0:NQB], v_sb[:, kb, h, :],
                            e_t[:, kb, sub * NQB:(sub + 1) * NQB],
                            start=(kb == 0), stop=(kb == KB - 1),
                        )

                def emit_ctx_tail(i):
                    h, bp = units[i]
                    pc = c_tiles[i]
                    # den row -> bf16 SBUF (ACT), broadcast to 64 partitions on
                    # the PE (ones ⊗ den), reciprocal + multiply on 64-lane DVE.
                    denb = npool.tile([1, NQW], BF16, tag="den")
                    nc.scalar.activation(
                        denb[:].rearrange("p (s q) -> p s q", q=NQB),
                        pc[HD:HD + 1, :, 0:NQB], AF.Copy)
                    rbps = sps.tile([P, 2, 512], F32, tag="sc", name=f"rb{i}")
                    for sub in range(2):
                        nc.tensor.matmul(rbps[0:HD, sub, 0:NQB], ones_sb[:],
                                         denb[:, sub * NQB:(sub + 1) * NQB],
                                         start=True, stop=True)
                    rb = npool.tile([HD, NQW], F32, tag="rb")
                    rb3 = rb[:].rearrange("p (s q) -> p s q", q=NQB)
                    nc.vector.reciprocal_approx_fast(rb3, rbps[0:HD, :, 0:NQB])
                    dst = (ctxa_sb[HD * h:HD * (h + 1), bp, 0:NQW] if h < 2
                           else ctxb_sb[:, bp, 0:NQW])
                    nc.vector.tensor_tensor(
                        dst.rearrange("p (s q) -> p s q", q=NQB),
                        pc[0:HD, :, 0:NQB], rb3, op=ALU.mult)

                ag_in = [dram.tile([HGD, NQW], BF16, name="agi0"),
                         dram.tile([HGD, NQW], BF16, name="agi1")]
                ag_out = dram.tile([2, TP, HGD, NQW], BF16)

                def emit_ag(bp):
                    # distinct DMA queues per AG -> independent completion
                    # semaphores, so AG(0) can start mid-attention instead of
                    # serializing both AGs at attention end.
                    dq = nc.scalar if bp == 0 else nc.sync
                    dq.dma_start(ag_in[bp][0:P, :], ctxa_sb[:, bp, :])
                    dq.dma_start(ag_in[bp][P:HGD, :], ctxb_sb[:, bp, :])
                    with nc.named_scope(f"ag{bp}"):
                        nc.gpsimd.collective_compute(
                            "AllGather", ALU.bypass, replica_groups=GROUPS,
                            ins=[ag_in[bp].opt()], outs=[ag_out[bp].opt()],
                        )

                for i in range(len(units) + 1):
                    if i < len(units):
                        e_tiles[i] = epool.tile([P, KB, NQW], BF16, tag="e",
                                                name=f"e{i}")
                    if i > 0:
                        c_tiles[i - 1] = cps.tile([P, 2, 512], F32, tag="c",
                                                  name=f"c{i-1}")
                    with nc.named_scope(f"unit{i}"):
                        for kb in range(KB):
                            if i < len(units):
                                emit_scores_kb(i, kb)
                            if i == 0:
                                emit_vproj(kb)
                                if kb < len(proj_rest):
                                    emit_proj_piece(*proj_rest[kb])
                            if i > 0:
                                emit_ctx_kb(i - 1, kb)
                    if i == 0:
                        emit_uo()
                    if i > 0:
                        emit_ctx_tail(i - 1)
                        del e_tiles[i - 1]
                        if (i - 1) % HG == HG - 1:
                            emit_ag((i - 1) // HG)
                attn_ctx.close()

            # ================= gather ctx for this core's block =================
            with tc.tile_critical():
                with nc.gpsimd.register("qx") as qx_reg:
                    nc.gpsimd.reg_load(qx_reg, qoff_sb[0:1, 0:1])
                    qx_v = nc.gpsimd.snap(qx_reg)
                with nc.gpsimd.register("qi") as qi_reg:
                    nc.gpsimd.reg_load(qi_reg, qoff_sb[0:1, 1:2])
                    qi_v = nc.gpsimd.snap(qi_reg)

            ag_r = (ag_out.rearrange("x g d q -> x (g d) q")
                    .rearrange("x (kt p) q -> p kt x q", p=P))
            ctxg_sb = qkv.tile([P, KT, NQB], BF16)
            with nc.named_scope("gather"):
                nc.gpsimd.dma_start(
                    ctxg_sb[:], ag_r[:, :, bass.ds(qx_v, 1), bass.ds(qi_v, NQB)])

            # ================= out dense + residual + LayerNorm =================
            with tc.tile_pool(name="ops", bufs=2, space="PSUM") as ops, \
                 tc.tile_pool(name="lnp", bufs=1) as lnp:
                uoB = uoB_sb
                h_all = lnp.tile([P, NST, H], F32)
                mu_all = lnp.tile([P, NST], F32)
                sq_all = lnp.tile([P, NST], F32)
                var_all = lnp.tile([P, NST], F32)
                sq_tmp = lnp.tile([P, H], F32, bufs=2)
                for st in range(NST):
                    r0 = st * P
                    vsz = max(0, min(P, NQB - r0))   # valid-dense rows in this tile
                    if vsz > 0:
                        ps = ops.tile([P, H], F32, tag="od", name=f"od{st}")
                        for kt in range(KT):
                            lhsT = ctxg_sb[:, kt, r0:r0 + vsz]
                            nc.tensor.matmul(ps[0:vsz, 0:512], lhsT,
                                             wo_sb[:, kt, 0:512],
                                             start=(kt == 0), stop=(kt == KT - 1))
                            nc.tensor.matmul(ps[0:vsz, 512:H], lhsT,
                                             wo_sb[:, kt, 512:H],
                                             start=(kt == 0), stop=(kt == KT - 1))
                        psc = lnp.tile([P, H], F32, tag="psc", bufs=2,
                                       name=f"psc{st}")
                        nc.scalar.activation(psc[0:vsz], ps[0:vsz], AF.Identity,
                                             accum_out=mu_all[0:vsz, st:st + 1])
                        nc.vector.tensor_tensor(h_all[0:vsz, st, :], psc[0:vsz],
                                                xres_sb[0:vsz, st, :], op=ALU.add)
                    # invalid-query rows: h = xres + uo (partition-aligned chunks)
                    s = vsz
                    while s < P:
                        m = 128 if s == 0 else (64 if s % 64 == 0 else 32)
                        e = min(s + m, P)
                        nc.vector.scalar_tensor_tensor(
                            out=h_all[s:e, st, :], in0=xres_sb[s:e, st, :],
                            scalar=1.0, in1=uoB[s:e, :],
                            op0=ALU.mult, op1=ALU.add,
                            accum_out=mu_all[s:e, st:st + 1])
                        s = e
                    # sum of squares now (pipelines with next tile's out-dense);
                    # var = E[h^2] - mu^2 later (mu ~ 0, no cancellation risk)
                    nc.scalar.activation(sq_tmp[:], h_all[:, st, :], AF.Square,
                                         accum_out=sq_all[:, st:st + 1])

                # mu = (accum + xsum)/H ; var = sumsq/H - mu^2
                nc.vector.tensor_tensor(mu_all[:], mu_all[:], xsum_sb[:], op=ALU.add)
                nc.vector.tensor_scalar_mul(mu_all[:], mu_all[:], 1.0 / H)
                negmu = lnp.tile([P, NST], F32)
                nc.vector.tensor_scalar_mul(negmu[:], mu_all[:], -1.0)
                nc.vector.scalar_tensor_tensor(
                    out=var_all[:], in0=mu_all[:], scalar=-1.0, in1=mu_all[:],
                    op0=ALU.mult, op1=ALU.mult)
                nc.vector.scalar_tensor_tensor(
                    out=var_all[:], in0=sq_all[:], scalar=1.0 / H, in1=var_all[:],
                    op0=ALU.mult, op1=ALU.add)
                nc.vector.tensor_scalar_add(var_all[:], var_all[:], EPS)
                # rstd = 1/sqrt(var) with one Newton step
                std0 = lnp.tile([P, NST], F32)
                nc.scalar.activation(std0[:], var_all[:], AF.Sqrt)
                y0 = lnp.tile([P, NST], F32)
                nc.vector.reciprocal(y0[:], std0[:])
                t0 = lnp.tile([P, NST], F32)
                nc.vector.tensor_tensor(t0[:], y0[:], y0[:], op=ALU.mult)
                nc.vector.tensor_tensor(t0[:], t0[:], var_all[:], op=ALU.mult)
                nc.vector.tensor_scalar_mul(t0[:], t0[:], -0.5)
                nc.vector.tensor_scalar_add(t0[:], t0[:], 1.5)
                rstd = lnp.tile([P, NST], F32)
                nc.vector.tensor_tensor(rstd[:], y0[:], t0[:], op=ALU.mult)

                for st in range(NST):
                    hc = lnp.tile([P, H], F32, tag="hc", bufs=2, name=f"hc{st}")
                    nc.scalar.activation(hc[:], h_all[:, st, :], AF.Identity,
                                         bias=negmu[:, st:st + 1])
                    o_sb = lnp.tile([P, H], F32, tag="o", bufs=2, name=f"o{st}")
                    nc.vector.scalar_tensor_tensor(
                        out=o_sb[:], in0=hc[:], scalar=rstd[:, st:st + 1],
                        in1=lng_sb[:], op0=ALU.mult, op1=ALU.mult)
                    nc.vector.tensor_tensor(o_sb[:], o_sb[:], lnb_sb[:], op=ALU.add)
                    nc.sync.dma_start(out_d[st * P:(st + 1) * P, :], o_sb[:])

    nc.compile()
    return nc


def _geometry(am):
    valid = am >= 0
    vidx = [np.where(valid[b])[0] for b in range(B)]
    bidx = {}
    iidx = {}
    for b in range(B):
        for g in range(TP):
            lo, hi = SQ * g, SQ * (g + 1)
            m = (vidx[b] >= lo) & (vidx[b] < hi)
            bidx[(b, g)] = vidx[b][m]
            inv = np.where(~valid[b, lo:hi])[0] + lo
            iidx[(b, g)] = inv
    maxv = max(len(v) for v in bidx.values())
    maxi = max(len(v) for v in iidx.values())
    # block padding must keep the 4-block compact axis a multiple of 128
    NQB = int(-(-max(maxv, 1) // 32) * 32)
    need = NQB + int(-(-max(maxi, 1) // 32) * 32)
    NROW = int(-(-need // P) * P)
    return vidx, bidx, iidx, NQB, NROW


def _prep_inputs(inputs, geom):
    vidx, bidx, iidx, NQB, NROW = geom
    NQA = 4 * NQB
    hs = np.asarray(inputs["hidden_states"], dtype=np.float32)
    Wq = np.asarray(inputs["Wq"], dtype=np.float32)
    Wk = np.asarray(inputs["Wk"], dtype=np.float32)
    Wv = np.asarray(inputs["Wv"], dtype=np.float32)
    Wo = np.asarray(inputs["Wo"], dtype=np.float32)
    bq = np.asarray(inputs["bq"], dtype=np.float32)
    bk = np.asarray(inputs["bk"], dtype=np.float32)
    bv = np.asarray(inputs["bv"], dtype=np.float32)
    bo = np.asarray(inputs["bo"], dtype=np.float32)
    lng = np.asarray(inputs["ln_gamma"], dtype=np.float32)
    lnb = np.asarray(inputs["ln_beta"], dtype=np.float32)

    wo_bf = Wo.astype(ml_dtypes.bfloat16)
    wv_bf = Wv.astype(ml_dtypes.bfloat16)
    bvf_rep = np.ascontiguousarray(bv.reshape(KT, P).T.astype(np.float32))
    lng_rep = np.ascontiguousarray(np.broadcast_to(lng, (P, H)))
    lnb_rep = np.ascontiguousarray(np.broadcast_to(lnb, (P, H)))

    # per-batch compacted tensors: one block-padded axis for keys AND queries
    xkt = []
    xbart = []
    mkb = []
    for b in range(B):
        xk = np.zeros((H, NQA), dtype=ml_dtypes.bfloat16)
        mk = np.full(NQA, BIGNEG, dtype=np.float32)
        for g in range(TP):
            bi = bidx[(b, g)]
            xk[:, NQB * g:NQB * g + len(bi)] = \
                hs[b].T[:, bi].astype(ml_dtypes.bfloat16)
            mk[NQB * g:NQB * g + len(bi)] = 0.0
        xkt.append(xk)
        mkb.append(mk)
        xbart.append(np.ascontiguousarray(
            hs[b].mean(axis=0).reshape(KT, P).T.astype(ml_dtypes.bfloat16)))

    in_maps = []
    for c in range(NCORES):
        b, g = c // TP, c % TP
        cs = slice(HGD * g, HGD * (g + 1))
        bi = bidx[(b, g)]
        ii = iidx[(b, g)]
        xres = np.zeros((NROW, H), dtype=np.float32)
        xres[0:len(bi)] = hs[b, bi] + bo
        xres[NQB:NQB + len(ii)] = hs[b, ii] + bo
        xsum = np.zeros(NROW, dtype=np.float32)
        xsum[0:len(bi)] = xres[0:len(bi)].sum(axis=1)
        in_maps.append({
            "xkt": xkt[b],
            "xbart": xbart[b],
            "xres": xres,
            "wq": np.ascontiguousarray(Wq[:, cs]).astype(ml_dtypes.bfloat16),
            "wk": np.ascontiguousarray(Wk[:, cs]).astype(ml_dtypes.bfloat16),
            "wv": np.ascontiguousarray(Wv[:, cs]).astype(ml_dtypes.bfloat16),
            "bq": np.ascontiguousarray(bq[cs]),
            "bk": np.ascontiguousarray(bk[cs]),
            "bvr": np.ascontiguousarray(np.broadcast_to(bv[cs], (P, HGD))),
            "wvf": wv_bf,
            "bvf": bvf_rep,
            "wo": np.ascontiguousarray(wo_bf),
            "mkb": mkb[b],
            "lng": lng_rep,
            "lnb": lnb_rep,
            "xsum": xsum,
            "qoff": np.array([[g // 2, (g % 2) * NQB]], dtype=np.uint32),
        })
    return in_maps


def _run(inputs, trace=False, trace_cores=None):
    am = np.asarray(inputs["attention_mask"], dtype=np.float32)
    geom = _geometry(am)
    _, bidx, iidx, NQB, NROW = geom
    key = (NQB, NROW)
    if key not in _cache:
        _cache[key] = build(*key)
    nc = _cache[key]
    in_maps = _prep_inputs(inputs, geom)
    res = run_bass_kernel_spmd(
        nc, in_maps, list(range(NCORES)), trace=trace,
        trace_cores=trace_cores,
    )
    out = np.empty((B, S, H), dtype=np.float32)
    for c in range(NCORES):
        b, g = c // TP, c % TP
        r = res.results[c]["out"]
        bi = bidx[(b, g)]
        ii = iidx[(b, g)]
        out[b, bi] = r[0:len(bi)]
        out[b, ii] = r[NQB:NQB + len(ii)]
    return out, res


def kernel(**inputs) -> np.ndarray:
    out, _ = _run(inputs)
    return out
